# revision 13
# baseline (speedup 1.0000x reference)
"""Causal self-attention (B=4, T=1024, D=1024, H=16) on 8 Trainium2 NeuronCores.

Sharding: heads 2c,2c+1 -> core c (head/tensor parallel). bf16 operands with
fp32 PSUM accumulation throughout.

Bias algebra (host-side folds): softmax is invariant to per-query constants,
so the k bias drops entirely; the q bias only matters through bq.k, kept by
adding bq to q during the PSUM->SBUF copy; the v bias passes through the
softmax average and folds into the output-projection bias; the 1/sqrt(dh)
scale folds into Wq. On-device the only bias work left is the q-copy add and
the key-mask row.

Per-batch AllToAll (4 small exchanges) redistributes attention outputs from
head-sharding to token-sharding; each core projects its 128 tokens per batch
with the full Wproj. A tiny alignment AllToAll at kernel start carries the
key-mask biases and absorbs cross-core launch skew before the first real
exchange. Output projection for batch b is emitted after batch b+1's
attention so its matmuls never head-of-line-block batch work on the PE.
"""
import numpy as np

B, T, D, H = 4, 1024, 1024, 16
DH = D // H  # 64
NC = 8
HPC = H // NC  # 2 heads per core

_CACHE = {}


def _build():
    import concourse.mybir as mybir
    import concourse.tile as tile
    from concourse import bacc

    BF16 = mybir.dt.bfloat16
    F32 = mybir.dt.float32
    EXP = mybir.ActivationFunctionType.Exp
    MULT = mybir.AluOpType.mult
    ADD = mybir.AluOpType.add

    nc = bacc.Bacc("TRN2", target_bir_lowering=False, debug=False, num_devices=NC)

    xt_d = nc.dram_tensor("xt", [B, 8, 128, T], BF16, kind="ExternalInput").ap()
    wqkv_d = nc.dram_tensor("wqkv", [8, 128, 3 * 128], BF16, kind="ExternalInput").ap()
    bq_d = nc.dram_tensor("bq", [64, HPC], F32, kind="ExternalInput").ap()
    biask_d = nc.dram_tensor("biask", [B, T], BF16, kind="ExternalInput").ap()
    wproj_d = nc.dram_tensor("wproj", [8, 128, D], BF16, kind="ExternalInput").ap()
    biasp_d = nc.dram_tensor("biasp", [128, D], BF16, kind="ExternalInput").ap()
    ident_d = nc.dram_tensor("ident", [128, 128], BF16, kind="ExternalInput").ap()
    tri_d = nc.dram_tensor("tri", [128, 128], BF16, kind="ExternalInput").ap()
    out_d = nc.dram_tensor("out", [B * 128, D], F32, kind="ExternalOutput").ap()

    with tile.TileContext(nc) as tc:
        with (
            tc.tile_pool(name="consts", bufs=1) as cpool,
            tc.tile_pool(name="xt", bufs=2) as xt_pool,
            tc.tile_pool(name="vt", bufs=2) as vt_pool,
            tc.tile_pool(name="att", bufs=10) as att_pool,
            tc.tile_pool(name="fin", bufs=2) as fin_pool,
            tc.tile_pool(name="nrm", bufs=2) as nrm_pool,
            tc.tile_pool(name="rcv", bufs=2) as rcv_pool,
            tc.tile_pool(name="ysb", bufs=2) as y_pool,
            tc.tile_pool(name="mmps", bufs=2, space="PSUM") as mm_ps,
            tc.tile_pool(name="sps", bufs=2, space="PSUM") as s_ps_pool,
            tc.tile_pool(name="tpps", bufs=1, space="PSUM") as tp_ps_pool,
            tc.tile_pool(name="ops", bufs=3, space="PSUM") as o_ps_pool,
            tc.tile_pool(name="dram", bufs=1, space="DRAM") as dram,
        ):
            # ---- collective buffers ----
            GW = [384, 128]  # group A: batches 0-2, group B: batch 3
            a2a_in = [
                dram.tile([8, 128, GW[g]], BF16, name=f"a2a_in{g}", tag=f"a2a_in{g}")
                for g in range(2)
            ]
            a2a_out = [
                dram.tile([8, 128, GW[g]], BF16, name=f"a2a_out{g}", tag=f"a2a_out{g}")
                for g in range(2)
            ]

            # ---- constants / weights (critical-path first: wq, then x) ----
            wq_sb = cpool.tile([128, 8, 384], BF16, name="wq", tag="wq")
            nc.sync.dma_start(wq_sb[:], wqkv_d.rearrange("a b c -> b a c"))
            biask_sb = cpool.tile([1, B * T], BF16, name="biask", tag="biask")
            tri = cpool.tile([128, 128], BF16, name="tri", tag="tri")
            ident = cpool.tile([128, 128], BF16, name="ident", tag="ident")
            bq_sb = cpool.tile([64, HPC], F32, name="bq", tag="bq")
            wp_sb = cpool.tile([128, 8, D], BF16, name="wp", tag="wp")
            biasp = cpool.tile([128, D], BF16, name="biasp", tag="biasp")

            # persistent q/k tiles (one extra contraction row) and v tiles
            # (ones columns for the softmax denominator), double-buffered by
            # batch parity
            qt = [[None, None], [None, None]]
            kt = [[None, None], [None, None]]
            for par in range(2):
                for h in range(HPC):
                    q_t = cpool.tile([65, T], BF16, name=f"qt{par}{h}", tag=f"qt{par}{h}")
                    nc.gpsimd.memset(q_t[64:65, :], 1.0)
                    qt[par][h] = q_t
                    kt[par][h] = cpool.tile(
                        [65, T], BF16, name=f"kt{par}{h}", tag=f"kt{par}{h}"
                    )
            # v tiles: per head, 64 ones-columns then 64 feature columns.
            # The ones block makes the AV matmul emit 64 replicated
            # denominator rows (matmul cost depends only on the free dim),
            # so normalization needs no partition broadcast.
            v_sb = [[None] * 8, [None] * 8]
            for par in range(2):
                for kb in range(8):
                    v_t = cpool.tile(
                        [128, 320], BF16, name=f"v{par}{kb}", tag=f"v{par}{kb}"
                    )
                    nc.gpsimd.memset(v_t[:, 0:64], 1.0)
                    nc.gpsimd.memset(v_t[:, 128:192], 1.0)
                    v_sb[par][kb] = v_t

            xt_sb = [
                cpool.tile([128, 8, T], BF16, name=f"xt{par}", tag=f"xt{par}")
                for par in range(2)
            ]
            # first token-half of batch 0 arrives per d-block so the first
            # qkv accumulation group paces with the DMA stream
            for i in range(8):
                nc.sync.dma_start(xt_sb[0][:, i, 0:512], xt_d[0, i, :, 0:512])
            nc.sync.dma_start(
                xt_sb[0][:, :, 512:T], xt_d[0].rearrange("a b c -> b a c")[:, :, 512:T]
            )
            nc.sync.dma_start(biask_sb[:], biask_d.rearrange("a b -> (a b)"))
            nc.sync.dma_start(tri[:], tri_d[:])
            nc.sync.dma_start(ident[:], ident_d[:])
            nc.sync.dma_start(bq_sb[:], bq_d[:])

            def proj_group(g):
                """Output projection for this core's rows of group g."""
                recv = rcv_pool.tile(
                    [128, 8, GW[g]], BF16, name=f"recv{g}", tag=f"recv{g}"
                )
                # data-dependency gate on batch 3: keeps the scheduler from
                # ordering proj work ahead of batch work on the engine queues
                nc.gpsimd.tensor_copy(recv[0:1, 0, 0:2], fin_last[0:1, 0:2])
                nc.sync.dma_start(recv[:], a2a_out[g].rearrange("c p f -> p c f"))
                for tb in range(GW[g] // 128):
                    b = 3 * g + tb
                    y_ps = [
                        mm_ps.tile([128, 512], F32, name="mm", tag="mm")
                        for _ in range(2)
                    ]
                    for c in range(8):
                        for ch in range(2):
                            nc.tensor.matmul(
                                y_ps[ch][:],
                                recv[:, c, tb * 128 : (tb + 1) * 128],
                                wp_sb[:, c, ch * 512 : (ch + 1) * 512],
                                start=(c == 0),
                                stop=(c == 7),
                            )
                    y_sb = y_pool.tile([128, D], F32, name="ysb", tag="ysb")
                    for ch in range(2):
                        csl = slice(ch * 512, (ch + 1) * 512)
                        nc.vector.tensor_tensor(
                            out=y_sb[:, csl], in0=y_ps[ch][:], in1=biasp[:, csl], op=ADD
                        )
                    nc.sync.dma_start(out_d[b * 128 : (b + 1) * 128, :], y_sb[:])

            for b in range(B):
                par = b % 2
                if b == 0:  # overlap heavy proj-weight loads with batch 0
                    nc.sync.dma_start(wp_sb[:], wproj_d.rearrange("a b c -> b a c"))
                    nc.sync.dma_start(biasp[:], biasp_d[:])
                if b < B - 1:
                    nc.sync.dma_start(
                        xt_sb[1 - par][:], xt_d[b + 1].rearrange("a b c -> b a c")
                    )

                # ---- qkv projections (transposed layout [feature, token]) --
                vt = vt_pool.tile([128, T], BF16, name="vt", tag="vt")
                for fb in range(3):
                    for ch in range(2):
                        csl = slice(ch * 512, (ch + 1) * 512)
                        ps = mm_ps.tile([128, 512], F32, name="mm", tag="mm")
                        for i in range(8):
                            nc.tensor.matmul(
                                ps[:],
                                wq_sb[:, i, fb * 128 : (fb + 1) * 128],
                                xt_sb[par][:, i, csl],
                                start=(i == 0),
                                stop=(i == 7),
                            )
                        if fb == 0:  # q: bias-add folded into the copy
                            for h in range(HPC):
                                nc.vector.tensor_scalar_add(
                                    qt[par][h][0:64, csl],
                                    ps[h * 64 : (h + 1) * 64, :],
                                    bq_sb[:, h : h + 1],
                                )
                        elif fb == 1:  # k: plain copy (bias dropped)
                            for h in range(HPC):
                                nc.scalar.copy(
                                    kt[par][h][0:64, csl], ps[h * 64 : (h + 1) * 64, :]
                                )
                        else:  # v
                            nc.scalar.copy(vt[:, csl], ps[:])
                # mask row
                for h in range(HPC):
                    nc.vector.tensor_copy(
                        kt[par][h][64:65, :], biask_sb[0:1, b * T : (b + 1) * T]
                    )
                # v transposed into [token, feature] blocks via PE
                for kb in range(8):
                    tp = tp_ps_pool.tile([128, 128], BF16, name="tp", tag="tp")
                    nc.tensor.transpose(tp[:], vt[:, kb * 128 : (kb + 1) * 128], ident[:])
                    # feature columns 64:128 (head 0) and 192:256 (head 1)
                    dst = v_sb[par][kb][:, 64:320].rearrange(
                        "p (c f) -> p c f", c=2, f=128
                    )[:, :, 0:64]
                    nc.vector.tensor_copy(dst, tp[:])

                # ---- attention per head ----
                fin = fin_pool.tile([128, T], BF16, name="fin", tag="fin")
                for h in range(HPC):
                    # scores + exp for all key blocks first: PE runs the score
                    # matmuls back-to-back while Act exps trail behind
                    att_t = []
                    for kb in range(8):
                        k0 = kb * 128
                        width = T - k0
                        att = att_pool.tile([128, T], BF16, name="att", tag="att")
                        for off in range(0, width, 512):
                            w = min(512, width - off)
                            s_ps = s_ps_pool.tile([128, 512], F32, name="s", tag="s")
                            nc.tensor.matmul(
                                s_ps[:, 0:w],
                                kt[par][h][:, k0 : k0 + 128],
                                qt[par][h][:, k0 + off : k0 + off + w],
                                start=True,
                                stop=True,
                            )
                            nc.scalar.activation(att[:, off : off + w], s_ps[:, 0:w], EXP)
                        nc.vector.tensor_tensor(
                            out=att[:, 0:128], in0=att[:, 0:128], in1=tri[:], op=MULT
                        )
                        att_t.append(att)
                    # AV accumulation (row 0 of o_ps is the denominator)
                    o_ps = [
                        o_ps_pool.tile([128, 512], F32, name="o_ps", tag="o_ps")
                        for _ in range(2)
                    ]
                    for kb in range(8):
                        k0 = kb * 128
                        for ch in range(2):
                            lo = max(k0, ch * 512)
                            hi = (ch + 1) * 512
                            if lo >= hi:
                                continue
                            nc.tensor.matmul(
                                o_ps[ch][:, lo - ch * 512 : hi - ch * 512],
                                v_sb[par][kb][:, h * 128 : (h + 1) * 128],
                                att_t[kb][:, lo - k0 : hi - k0],
                                start=(kb == 0),
                                stop=(kb == (3 if ch == 0 else 7)),
                            )
                    # normalize: rows 0:64 of o_ps are 64 copies of the
                    # denominator, rows 64:128 the features
                    for ch in range(2):
                        csl = slice(ch * 512, (ch + 1) * 512)
                        recip = nrm_pool.tile([64, 512], F32, name="recip", tag="recip")
                        nc.vector.reciprocal_approx_fast(recip[:], o_ps[ch][0:64, :])
                        nc.vector.tensor_tensor(
                            out=fin[h * 64 : (h + 1) * 64, csl],
                            in0=o_ps[ch][64:128, :],
                            in1=recip[:],
                            op=MULT,
                        )
                # stage this batch's attention outputs; exchange per group
                g, half = (0, b) if b < 3 else (1, 0)
                nc.sync.dma_start(
                    a2a_in[g][:, :, half * 128 : half * 128 + 128].rearrange(
                        "c p f -> p c f"
                    ),
                    fin[:].rearrange("p (c f) -> p c f", c=8, f=128),
                )
                if b == 2 or b == 3:
                    nc.gpsimd.collective_compute(
                        "AllToAll",
                        mybir.AluOpType.bypass,
                        replica_groups=[list(range(NC))],
                        ins=[a2a_in[g][:].opt()],
                        outs=[a2a_out[g][:].opt()],
                    )
                if b == B - 1:
                    fin_last = fin
            # push priorities far past the batch pipeline so no proj work is
            # scheduled ahead of batch work on any engine queue (head-of-line)
            tc.cur_priority += 100000
            proj_group(0)
            tc.cur_priority += 100000
            proj_group(1)

    nc.compile()
    return nc


def _get_nc():
    if "nc" not in _CACHE:
        _CACHE["nc"] = _build()
    return _CACHE["nc"]


def kernel(x, Wqkv, bqkv, Wproj, bproj, mask):
    from concourse.bass_utils import run_bass_kernel_spmd
    import ml_dtypes

    bf16 = ml_dtypes.bfloat16
    x = np.asarray(x, dtype=np.float32)
    Wqkv = np.asarray(Wqkv, dtype=np.float32)
    bqkv = np.asarray(bqkv, dtype=np.float32)
    Wproj = np.asarray(Wproj, dtype=np.float32)
    bproj = np.asarray(bproj, dtype=np.float32)
    mask = np.asarray(mask)

    nc = _get_nc()

    xt = np.ascontiguousarray(x.transpose(0, 2, 1)).reshape(B, 8, 128, T)
    biask = np.where(mask == 0, np.float32(-30000.0), np.float32(0.0))
    # v bias passes through the softmax average: fold it into the proj bias
    bproj_eff = bproj + bqkv[2 * D : 3 * D] @ Wproj
    biasp = np.broadcast_to(bproj_eff, (128, D))
    ident = np.eye(128, dtype=np.float32)
    tri = np.triu(np.ones((128, 128), np.float32))

    in_maps = []
    for c in range(NC):
        cols = slice(c * 128, (c + 1) * 128)  # this core's head features
        wq = Wqkv[:, 0:D][:, cols] * 0.125  # score scale folded into Wq
        wk = Wqkv[:, D : 2 * D][:, cols]
        wv = Wqkv[:, 2 * D : 3 * D][:, cols]
        w_local = np.concatenate([wq, wk, wv], axis=1).reshape(8, 128, 384)
        bq = (bqkv[0:D][cols] * 0.125).reshape(HPC, 64).T  # [64, HPC]
        in_maps.append(
            {
                "xt": xt.astype(bf16),
                "wqkv": np.ascontiguousarray(w_local).astype(bf16),
                "bq": np.ascontiguousarray(bq),
                "biask": biask.astype(bf16),
                "wproj": Wproj.reshape(8, 128, D).astype(bf16),
                "biasp": biasp.astype(bf16),
                "ident": ident.astype(bf16),
                "tri": tri.astype(bf16),
            }
        )

    res = run_bass_kernel_spmd(nc, in_maps, core_ids=list(range(NC)))
    # core c group g rows: tokens [c*128, (c+1)*128) of batch g
    y = np.empty((B, T, D), np.float32)
    for c in range(NC):
        oc = res.results[c]["out"]
        for g in range(B):
            y[g, c * 128 : (c + 1) * 128] = oc[g * 128 : (g + 1) * 128]
    return y


# revision 14
# speedup vs baseline: 1.1384x; 1.1384x over previous
"""Causal self-attention (B=4, T=1024, D=1024, H=16) on 8 Trainium2 NeuronCores.

Sharding: heads 2c,2c+1 -> core c (head/tensor parallel). bf16 operands with
fp32 PSUM accumulation throughout.

Bias algebra (host-side folds): softmax is invariant to per-query constants,
so the k bias drops entirely; the q bias only matters through bq.k, kept by
adding bq to q during the PSUM->SBUF copy; the v bias passes through the
softmax average and folds into the output-projection bias; the 1/sqrt(dh)
scale folds into Wq. On-device the only bias work left is the q-copy add and
the key-mask row.

Per-batch AllToAll (4 small exchanges) redistributes attention outputs from
head-sharding to token-sharding; each core projects its 128 tokens per batch
with the full Wproj. A tiny alignment AllToAll at kernel start carries the
key-mask biases and absorbs cross-core launch skew before the first real
exchange. Output projection for batch b is emitted after batch b+1's
attention so its matmuls never head-of-line-block batch work on the PE.
"""
import numpy as np

B, T, D, H = 4, 1024, 1024, 16
DH = D // H  # 64
NC = 8
HPC = H // NC  # 2 heads per core

_CACHE = {}


def _build():
    import concourse.mybir as mybir
    import concourse.tile as tile
    from concourse import bacc

    BF16 = mybir.dt.bfloat16
    F32 = mybir.dt.float32
    EXP = mybir.ActivationFunctionType.Exp
    MULT = mybir.AluOpType.mult
    ADD = mybir.AluOpType.add

    nc = bacc.Bacc("TRN2", target_bir_lowering=False, debug=False, num_devices=NC)

    xt_d = nc.dram_tensor("xt", [B, 8, 128, T], BF16, kind="ExternalInput").ap()
    wqkv_d = nc.dram_tensor("wqkv", [8, 128, 3 * 128], BF16, kind="ExternalInput").ap()
    bq_d = nc.dram_tensor("bq", [64, HPC], F32, kind="ExternalInput").ap()
    biask_d = nc.dram_tensor("biask", [B, T], BF16, kind="ExternalInput").ap()
    wproj_d = nc.dram_tensor("wproj", [8, 128, D], BF16, kind="ExternalInput").ap()
    biasp_d = nc.dram_tensor("biasp", [128, D], BF16, kind="ExternalInput").ap()
    ident_d = nc.dram_tensor("ident", [128, 128], BF16, kind="ExternalInput").ap()
    tri_d = nc.dram_tensor("tri", [128, 128], BF16, kind="ExternalInput").ap()
    out_d = nc.dram_tensor("out", [B * 128, D], F32, kind="ExternalOutput").ap()

    with tile.TileContext(nc) as tc:
        with (
            tc.tile_pool(name="consts", bufs=1) as cpool,
            tc.tile_pool(name="xt", bufs=2) as xt_pool,
            tc.tile_pool(name="vt", bufs=2) as vt_pool,
            tc.tile_pool(name="att", bufs=10) as att_pool,
            tc.tile_pool(name="fin", bufs=2) as fin_pool,
            tc.tile_pool(name="nrm", bufs=2) as nrm_pool,
            tc.tile_pool(name="rcv", bufs=2) as rcv_pool,
            tc.tile_pool(name="ysb", bufs=2) as y_pool,
            tc.tile_pool(name="mmps", bufs=2, space="PSUM") as mm_ps,
            tc.tile_pool(name="sps", bufs=2, space="PSUM") as s_ps_pool,
            tc.tile_pool(name="tpps", bufs=1, space="PSUM") as tp_ps_pool,
            tc.tile_pool(name="ops", bufs=3, space="PSUM") as o_ps_pool,
            tc.tile_pool(name="dram", bufs=1, space="DRAM") as dram,
        ):
            # ---- collective buffers ----
            a2a_in = [
                dram.tile([8, 128, 128], BF16, name=f"a2a_in{g}", tag=f"a2a_in{g}")
                for g in range(B)
            ]
            a2a_out = [
                dram.tile([8, 128, 128], BF16, name=f"a2a_out{g}", tag=f"a2a_out{g}")
                for g in range(B)
            ]

            # ---- constants / weights (critical-path first: wq, then x) ----
            wq_sb = cpool.tile([128, 8, 384], BF16, name="wq", tag="wq")
            nc.sync.dma_start(wq_sb[:], wqkv_d.rearrange("a b c -> b a c"))
            biask_sb = cpool.tile([1, B * T], BF16, name="biask", tag="biask")
            tri = cpool.tile([128, 128], BF16, name="tri", tag="tri")
            ident = cpool.tile([128, 128], BF16, name="ident", tag="ident")
            bq_sb = cpool.tile([64, HPC], F32, name="bq", tag="bq")
            wp_sb = cpool.tile([128, 8, D], BF16, name="wp", tag="wp")
            biasp = cpool.tile([128, D], BF16, name="biasp", tag="biasp")

            # persistent q/k tiles (one extra contraction row) and v tiles
            # (ones columns for the softmax denominator), double-buffered by
            # batch parity
            qt = [[None, None], [None, None]]
            kt = [[None, None], [None, None]]
            for par in range(2):
                for h in range(HPC):
                    q_t = cpool.tile([65, T], BF16, name=f"qt{par}{h}", tag=f"qt{par}{h}")
                    nc.gpsimd.memset(q_t[64:65, :], 1.0)
                    qt[par][h] = q_t
                    kt[par][h] = cpool.tile(
                        [65, T], BF16, name=f"kt{par}{h}", tag=f"kt{par}{h}"
                    )
            # v tiles: per head, 64 ones-columns then 64 feature columns.
            # The ones block makes the AV matmul emit 64 replicated
            # denominator rows (matmul cost depends only on the free dim),
            # so normalization needs no partition broadcast.
            v_sb = [[None] * 8, [None] * 8]
            for par in range(2):
                for kb in range(8):
                    v_t = cpool.tile(
                        [128, 320], BF16, name=f"v{par}{kb}", tag=f"v{par}{kb}"
                    )
                    nc.gpsimd.memset(v_t[:, 0:64], 1.0)
                    nc.gpsimd.memset(v_t[:, 128:192], 1.0)
                    v_sb[par][kb] = v_t

            xt_sb = [
                cpool.tile([128, 8, T], BF16, name=f"xt{par}", tag=f"xt{par}")
                for par in range(2)
            ]
            # first token-half of batch 0 arrives per d-block so the first
            # qkv accumulation group paces with the DMA stream
            for i in range(8):
                nc.sync.dma_start(xt_sb[0][:, i, 0:512], xt_d[0, i, :, 0:512])
            nc.sync.dma_start(
                xt_sb[0][:, :, 512:T], xt_d[0].rearrange("a b c -> b a c")[:, :, 512:T]
            )
            nc.sync.dma_start(biask_sb[:], biask_d.rearrange("a b -> (a b)"))
            nc.sync.dma_start(tri[:], tri_d[:])
            nc.sync.dma_start(ident[:], ident_d[:])
            nc.sync.dma_start(bq_sb[:], bq_d[:])

            def proj_group(g):
                """Output projection for this core's 128 rows of batch g."""
                recv = rcv_pool.tile(
                    [128, 8, 128], BF16, name=f"recv{g}", tag=f"recv{g}"
                )
                # data-dependency gate on batch 3: keeps the scheduler from
                # ordering proj work ahead of batch work on the engine queues
                nc.vector.tensor_copy(recv[0:1, 0, 0:2], fin_last[0:1, 0:2])
                nc.sync.dma_start(recv[:], a2a_out[g].rearrange("c p f -> p c f"))
                for tb in range(1):
                    b = g
                    y_ps = [
                        mm_ps.tile([128, 512], F32, name="mm", tag="mm")
                        for _ in range(2)
                    ]
                    for c in range(8):
                        for ch in range(2):
                            nc.tensor.matmul(
                                y_ps[ch][:],
                                recv[:, c, tb * 128 : (tb + 1) * 128],
                                wp_sb[:, c, ch * 512 : (ch + 1) * 512],
                                start=(c == 0),
                                stop=(c == 7),
                            )
                    y_sb = y_pool.tile([128, D], F32, name="ysb", tag="ysb")
                    for ch in range(2):
                        csl = slice(ch * 512, (ch + 1) * 512)
                        nc.vector.tensor_tensor(
                            out=y_sb[:, csl], in0=y_ps[ch][:], in1=biasp[:, csl], op=ADD
                        )
                    nc.sync.dma_start(out_d[b * 128 : (b + 1) * 128, :], y_sb[:])

            for b in range(B):
                par = b % 2
                if b == 0:  # overlap heavy proj-weight loads with batch 0
                    nc.sync.dma_start(wp_sb[:], wproj_d.rearrange("a b c -> b a c"))
                    nc.sync.dma_start(biasp[:], biasp_d[:])
                if b < B - 1:
                    nc.sync.dma_start(
                        xt_sb[1 - par][:], xt_d[b + 1].rearrange("a b c -> b a c")
                    )

                # ---- qkv projections (transposed layout [feature, token]) --
                vt = vt_pool.tile([128, T], BF16, name="vt", tag="vt")
                for fb in range(3):
                    for ch in range(2):
                        csl = slice(ch * 512, (ch + 1) * 512)
                        ps = mm_ps.tile([128, 512], F32, name="mm", tag="mm")
                        for i in range(8):
                            nc.tensor.matmul(
                                ps[:],
                                wq_sb[:, i, fb * 128 : (fb + 1) * 128],
                                xt_sb[par][:, i, csl],
                                start=(i == 0),
                                stop=(i == 7),
                            )
                        if fb == 0:  # q: bias-add folded into the copy
                            for h in range(HPC):
                                nc.vector.tensor_scalar_add(
                                    qt[par][h][0:64, csl],
                                    ps[h * 64 : (h + 1) * 64, :],
                                    bq_sb[:, h : h + 1],
                                )
                        elif fb == 1:  # k: plain copy (bias dropped)
                            for h in range(HPC):
                                nc.scalar.copy(
                                    kt[par][h][0:64, csl], ps[h * 64 : (h + 1) * 64, :]
                                )
                        else:  # v
                            nc.scalar.copy(vt[:, csl], ps[:])
                # mask row
                for h in range(HPC):
                    nc.vector.tensor_copy(
                        kt[par][h][64:65, :], biask_sb[0:1, b * T : (b + 1) * T]
                    )
                # v transposed into [token, feature] blocks via PE
                for kb in range(8):
                    tp = tp_ps_pool.tile([128, 128], BF16, name="tp", tag="tp")
                    nc.tensor.transpose(tp[:], vt[:, kb * 128 : (kb + 1) * 128], ident[:])
                    # feature columns 64:128 (head 0) and 192:256 (head 1)
                    dst = v_sb[par][kb][:, 64:320].rearrange(
                        "p (c f) -> p c f", c=2, f=128
                    )[:, :, 0:64]
                    nc.vector.tensor_copy(dst, tp[:])

                # ---- attention per head ----
                fin = fin_pool.tile([128, T], BF16, name="fin", tag="fin")
                for h in range(HPC):
                    # scores + exp for all key blocks first: PE runs the score
                    # matmuls back-to-back while Act exps trail behind
                    att_t = []
                    for kb in range(8):
                        k0 = kb * 128
                        width = T - k0
                        att = att_pool.tile([128, T], BF16, name="att", tag="att")
                        for off in range(0, width, 512):
                            w = min(512, width - off)
                            s_ps = s_ps_pool.tile([128, 512], F32, name="s", tag="s")
                            nc.tensor.matmul(
                                s_ps[:, 0:w],
                                kt[par][h][:, k0 : k0 + 128],
                                qt[par][h][:, k0 + off : k0 + off + w],
                                start=True,
                                stop=True,
                            )
                            nc.scalar.activation(att[:, off : off + w], s_ps[:, 0:w], EXP)
                        nc.vector.tensor_tensor(
                            out=att[:, 0:128], in0=att[:, 0:128], in1=tri[:], op=MULT
                        )
                        att_t.append(att)
                    # AV accumulation (row 0 of o_ps is the denominator)
                    o_ps = [
                        o_ps_pool.tile([128, 512], F32, name="o_ps", tag="o_ps")
                        for _ in range(2)
                    ]
                    for kb in range(8):
                        k0 = kb * 128
                        for ch in range(2):
                            lo = max(k0, ch * 512)
                            hi = (ch + 1) * 512
                            if lo >= hi:
                                continue
                            nc.tensor.matmul(
                                o_ps[ch][:, lo - ch * 512 : hi - ch * 512],
                                v_sb[par][kb][:, h * 128 : (h + 1) * 128],
                                att_t[kb][:, lo - k0 : hi - k0],
                                start=(kb == 0),
                                stop=(kb == (3 if ch == 0 else 7)),
                            )
                    # normalize: rows 0:64 of o_ps are 64 copies of the
                    # denominator, rows 64:128 the features
                    for ch in range(2):
                        csl = slice(ch * 512, (ch + 1) * 512)
                        recip = nrm_pool.tile([64, 512], F32, name="recip", tag="recip")
                        nc.vector.reciprocal_approx_fast(recip[:], o_ps[ch][0:64, :])
                        nc.vector.tensor_tensor(
                            out=fin[h * 64 : (h + 1) * 64, csl],
                            in0=o_ps[ch][64:128, :],
                            in1=recip[:],
                            op=MULT,
                        )
                # stage and exchange this batch's attention outputs
                nc.sync.dma_start(
                    a2a_in[b].rearrange("c p f -> p c f"),
                    fin[:].rearrange("p (c f) -> p c f", c=8, f=128),
                )
                nc.gpsimd.collective_compute(
                    "AllToAll",
                    mybir.AluOpType.bypass,
                    replica_groups=[list(range(NC))],
                    ins=[a2a_in[b][:].opt()],
                    outs=[a2a_out[b][:].opt()],
                )
                if b == B - 1:
                    fin_last = fin
            # push priorities far past the batch pipeline so no proj work is
            # scheduled ahead of batch work on any engine queue (head-of-line)
            for g in range(B):
                tc.cur_priority += 100000
                proj_group(g)

    nc.compile()
    return nc


def _get_nc():
    if "nc" not in _CACHE:
        _CACHE["nc"] = _build()
    return _CACHE["nc"]


def kernel(x, Wqkv, bqkv, Wproj, bproj, mask):
    from concourse.bass_utils import run_bass_kernel_spmd
    import ml_dtypes

    bf16 = ml_dtypes.bfloat16
    x = np.asarray(x, dtype=np.float32)
    Wqkv = np.asarray(Wqkv, dtype=np.float32)
    bqkv = np.asarray(bqkv, dtype=np.float32)
    Wproj = np.asarray(Wproj, dtype=np.float32)
    bproj = np.asarray(bproj, dtype=np.float32)
    mask = np.asarray(mask)

    nc = _get_nc()

    xt = np.ascontiguousarray(x.transpose(0, 2, 1)).reshape(B, 8, 128, T)
    biask = np.where(mask == 0, np.float32(-30000.0), np.float32(0.0))
    # v bias passes through the softmax average: fold it into the proj bias
    bproj_eff = bproj + bqkv[2 * D : 3 * D] @ Wproj
    biasp = np.broadcast_to(bproj_eff, (128, D))
    ident = np.eye(128, dtype=np.float32)
    tri = np.triu(np.ones((128, 128), np.float32))

    in_maps = []
    for c in range(NC):
        cols = slice(c * 128, (c + 1) * 128)  # this core's head features
        wq = Wqkv[:, 0:D][:, cols] * 0.125  # score scale folded into Wq
        wk = Wqkv[:, D : 2 * D][:, cols]
        wv = Wqkv[:, 2 * D : 3 * D][:, cols]
        w_local = np.concatenate([wq, wk, wv], axis=1).reshape(8, 128, 384)
        bq = (bqkv[0:D][cols] * 0.125).reshape(HPC, 64).T  # [64, HPC]
        in_maps.append(
            {
                "xt": xt.astype(bf16),
                "wqkv": np.ascontiguousarray(w_local).astype(bf16),
                "bq": np.ascontiguousarray(bq),
                "biask": biask.astype(bf16),
                "wproj": Wproj.reshape(8, 128, D).astype(bf16),
                "biasp": biasp.astype(bf16),
                "ident": ident.astype(bf16),
                "tri": tri.astype(bf16),
            }
        )

    res = run_bass_kernel_spmd(nc, in_maps, core_ids=list(range(NC)))
    # core c group g rows: tokens [c*128, (c+1)*128) of batch g
    y = np.empty((B, T, D), np.float32)
    for c in range(NC):
        oc = res.results[c]["out"]
        for g in range(B):
            y[g, c * 128 : (c + 1) * 128] = oc[g * 128 : (g + 1) * 128]
    return y
